# revision 1
# baseline (speedup 1.0000x reference)
"""Trainium2 Bass kernel for 3D neighborhood attention (NATTEN-style).

Sharding: H axis (32) split across 8 cores, 4 own rows + a 2-row halo each
side (host-padded to a uniform 8-row window; W padded by 2 each side). All
neighborhood masking is folded into the score matmul contraction via
indicator/penalty rows:
  scoresT[k,q] = k.T q + sum_r U[r,k] * V[r,q]
with U = key-side (d,h,w)-position indicators (0/1) and V = query-side
-30000 * (1 - valid) penalties, so exp() zeroes out-of-window keys with no
extra vector ops. Contraction K = 64 (head dim) + 4 + 8 + 52 = 128 exactly.

Layouts (chosen so every matmul operand is a contiguous SBUF slice):
  kT per head: [128, NV], free dim w-major: pos = w*32 + d*8 + h
  qT per head: [128, NQ], free dim wtile-major: pos = wt*256 + d*64 + h*16 + wl
  key chunk (wt, ci) = kT cols [512*wt + 128*ci, +128)  (w' quad, all d', h')
  query block wt     = qT cols [256*wt, +256)

Pipeline per core (one NEFF): QKV (f32r) -> scoresT (bf16, keys-major) ->
exp (ACT, masked keys -> 0) -> attn.T@[v|1] (bf16; col 64 of the rhs is ones,
giving softmax sums alongside the unnormalized output) -> reciprocal+scale
(DVE) -> PE transpose -> proj (bf16) -> strided DMA out.
"""
import numpy as np
import ml_dtypes

import concourse.bass as bass
import concourse.bacc as bacc
import concourse.mybir as mybir
from concourse.tile import TileContext
from concourse.bass_utils import run_bass_kernel_spmd

F32R = mybir.dt.float32r
BF16 = mybir.dt.bfloat16
F32 = mybir.dt.float32

NCORES = 8
D, H, W, C = 4, 32, 48, 256
HEADS, HD = 4, 64
KD, KH, KW = 3, 5, 5
SCALE = HD ** -0.5
BIG = 30000.0

HH = 8            # halo rows per core (padded-uniform)
WP = W + 4        # padded W
NV = D * HH * WP  # 1664 voxels per shard (with halo+pad)
NQ = D * 4 * W    # 768 own queries per core
NWT = 3           # w tiles of 16 queries
NCK = 5           # key chunks of 128 per w-tile (4 w' x 4 d x 8 h)

_CACHE = {}


def _build_program():
    nc = bacc.Bacc("TRN2", target_bir_lowering=False, debug=False,
                   num_devices=NCORES)
    xT_in = nc.declare_dram_parameter("xT", [C, NV], F32R, isOutput=False)
    xTq_in = nc.declare_dram_parameter("xTq", [C, NQ], F32R, isOutput=False)
    wq_in = nc.declare_dram_parameter("wq", [C, C], F32R, isOutput=False)
    wk_in = nc.declare_dram_parameter("wk", [C, C], F32R, isOutput=False)
    wv_in = nc.declare_dram_parameter("wv", [C, C], F32R, isOutput=False)
    wp_in = nc.declare_dram_parameter("wp", [C, C], F32R, isOutput=False)
    u_in = nc.declare_dram_parameter("u", [64, NV], F32R, isOutput=False)
    v_in = nc.declare_dram_parameter("vq", [64, NQ], F32R, isOutput=False)
    bqkv_in = nc.declare_dram_parameter("bqkv", [128, 6], F32, isOutput=False)
    bv_in = nc.declare_dram_parameter("bv", [1, C], F32, isOutput=False)
    bp_in = nc.declare_dram_parameter("bp", [1, C], F32, isOutput=False)
    ones1_in = nc.declare_dram_parameter("ones1", [1, 64], F32R, isOutput=False)
    ones60_in = nc.declare_dram_parameter("ones60", [128, 60], F32R, isOutput=False)
    y_out = nc.declare_dram_parameter("y", [NQ, C], F32, isOutput=True)
    v_dram = nc.dram_tensor("v_scratch", [NV, C], F32R)

    with TileContext(nc) as tc:
        with (
            nc.allow_low_precision(reason="float32r tiles are bit-identical to f32"),
            tc.tile_pool(name="const", bufs=1) as cp,
            tc.tile_pool(name="psA", bufs=2, space="PSUM") as psA,
            tc.tile_pool(name="psS", bufs=2, space="PSUM") as psS,
            tc.tile_pool(name="work", bufs=4) as wkp,
        ):
            # ---- constant / input loads ----
            xT = [cp.tile([128, NV], F32R, tag=f"xT{i}", name=f"xT{i}")
                  for i in range(2)]
            xTq = [cp.tile([128, NQ], F32R, tag=f"xTq{i}", name=f"xTq{i}")
                   for i in range(2)]
            for i in range(2):
                nc.sync.dma_start(out=xT[i][:], in_=xT_in[128 * i:128 * (i + 1), :])
                nc.sync.dma_start(out=xTq[i][:], in_=xTq_in[128 * i:128 * (i + 1), :])
            wq_t = [cp.tile([128, C], F32R, tag=f"wq{i}", name=f"wq{i}") for i in range(2)]
            wk_t = [cp.tile([128, C], F32R, tag=f"wk{i}", name=f"wk{i}") for i in range(2)]
            wv_t = [cp.tile([128, C], F32R, tag=f"wv{i}", name=f"wv{i}") for i in range(2)]
            for i in range(2):
                nc.sync.dma_start(out=wq_t[i][:], in_=wq_in[128 * i:128 * (i + 1), :])
                nc.sync.dma_start(out=wk_t[i][:], in_=wk_in[128 * i:128 * (i + 1), :])
                nc.sync.dma_start(out=wv_t[i][:], in_=wv_in[128 * i:128 * (i + 1), :])
            wp_t = [cp.tile([64, C], F32R, tag=f"wp{i}", name=f"wp{i}") for i in range(4)]
            for i in range(4):
                nc.sync.dma_start(out=wp_t[i][:], in_=wp_in[64 * i:64 * (i + 1), :])
            kT = [cp.tile([128, NV], F32R, tag=f"kT{h}", name=f"kTh{h}")
                  for h in range(HEADS)]
            qT = [cp.tile([128, NQ], F32R, tag=f"qT{h}", name=f"qTh{h}")
                  for h in range(HEADS)]
            for h in range(HEADS):
                nc.sync.dma_start(out=kT[h][64:128, :], in_=u_in[:])
                nc.sync.dma_start(out=qT[h][64:128, :], in_=v_in[:])
            bqkv = cp.tile([128, 6], F32)
            nc.sync.dma_start(out=bqkv[:], in_=bqkv_in[:])
            bv_row = cp.tile([1, C], F32)
            bp_row = cp.tile([1, C], F32)
            nc.sync.dma_start(out=bv_row[:], in_=bv_in[:])
            nc.sync.dma_start(out=bp_row[:], in_=bp_in[:])
            bv_b = cp.tile([128, C], F32)
            bp_b = cp.tile([128, C], F32)
            nc.gpsimd.partition_broadcast(bv_b[:], bv_row[:])
            nc.gpsimd.partition_broadcast(bp_b[:], bp_row[:])
            ones_c = cp.tile([1, 64], F32R)
            nc.sync.dma_start(out=ones_c[:], in_=ones1_in[:])
            v_gt = cp.tile([128, 260 * NWT * NCK], F32R, name="v_gt")
            dstc = v_gt[:].rearrange("p (i a b) -> p i a b",
                                     i=NWT * NCK, a=HEADS, b=65)[:, :, :, 64]
            nc.sync.dma_start(out=dstc, in_=ones60_in[:].rearrange(
                "p (i a) -> p i a", i=NWT * NCK, a=HEADS))

            # ---- QKV ----
            # v first (vox-major, all voxels) -> DRAM scratch -> gathers, so the
            # DRAM roundtrip overlaps the q/k matmuls below
            for t in range(NV // 128):
                ps = psA.tile([128, C], F32, tag="ps")
                for kc in range(2):
                    nc.tensor.matmul(ps[:], xT[kc][:, 128 * t:128 * (t + 1)],
                                     wv_t[kc][:], start=(kc == 0), stop=(kc == 1))
                vsb = wkp.tile([128, C], F32R, tag="vsb")
                nc.vector.tensor_tensor(out=vsb[:], in0=ps[:], in1=bv_b[:],
                                        op=mybir.AluOpType.add)
                nc.sync.dma_start(out=v_dram[128 * t:128 * (t + 1), :], in_=vsb[:])
            # gather v into key-chunk order (w', d, h), all heads in one DMA
            vv = v_dram.ap().rearrange("(d h w) c -> w d h c", d=D, h=HH, w=WP)
            for wt in range(NWT):
                for ci in range(NCK):
                    lo = 16 * wt + 4 * ci
                    base = 260 * (wt * NCK + ci)
                    for h in range(HEADS):
                        nc.sync.dma_start(
                            out=v_gt[:, base + 65 * h:base + 65 * h + 64],
                            in_=vv[lo:lo + 4, :, :, 64 * h:64 * (h + 1)])
            for m in range(2):          # output channel chunk (= head pair)
                # q: own voxels, from the query-ordered xTq copy (contiguous)
                for nn in range(2):
                    ps = psA.tile([128, 384], F32, tag="ps")
                    for kc in range(2):
                        nc.tensor.matmul(ps[:], wq_t[kc][:, 128 * m:128 * (m + 1)],
                                         xTq[kc][:, 384 * nn:384 * (nn + 1)],
                                         start=(kc == 0), stop=(kc == 1))
                    for hh in range(2):
                        nc.vector.tensor_scalar(
                            out=qT[2 * m + hh][0:64, 384 * nn:384 * (nn + 1)],
                            in0=ps[64 * hh:64 * (hh + 1), :],
                            scalar1=bqkv[64 * hh:64 * (hh + 1), m:m + 1],
                            scalar2=None, op0=mybir.AluOpType.add)
                # k: all voxels; copyback scatters (h,w)-order into w-major kT
                for nn in range(4):     # d plane (vox quarter, 416 each)
                    ps = psA.tile([128, 416], F32, tag="ps")
                    for kc in range(2):
                        nc.tensor.matmul(
                            ps[:], wk_t[kc][:, 128 * m:128 * (m + 1)],
                            xT[kc][:, 416 * nn:416 * (nn + 1)],
                            start=(kc == 0), stop=(kc == 1))
                    for hh in range(2):
                        dst = kT[2 * m + hh][0:64, :].rearrange(
                            "p (w d h) -> p h w d", w=WP, d=D, h=HH)[:, :, :, nn]
                        src = ps[64 * hh:64 * (hh + 1), :].rearrange(
                            "p (h w) -> p h w", h=HH, w=WP)
                        nc.vector.tensor_scalar(
                            out=dst, in0=src,
                            scalar1=bqkv[64 * hh:64 * (hh + 1), 2 + m:3 + m],
                            scalar2=None, op0=mybir.AluOpType.add)

            # ---- attention + proj ----
            yv = y_out.ap().rearrange("(wt d h wl) c -> wt d h wl c",
                                      wt=NWT, d=D, h=4, wl=16)
            for wt in range(NWT):
                aoT = [wkp.tile([64, C], F32R, tag=f"aoT{h}", name=f"aoTt{h}")
                       for h in range(HEADS)]
                for h in range(HEADS):
                    ps_s = psS.tile([128, 1280], F32, tag="ps_s")
                    for ci in range(NCK):
                        nc.tensor.matmul(
                            ps_s[:, 256 * ci:256 * (ci + 1)],
                            kT[h][:, 512 * wt + 128 * ci:512 * wt + 128 * (ci + 1)],
                            qT[h][:, 256 * wt:256 * (wt + 1)],
                            start=True, stop=True)
                    ex = wkp.tile([128, 1280], F32R, tag="ex")
                    nc.scalar.activation(ex[:], ps_s[:],
                                         mybir.ActivationFunctionType.Exp)
                    # unnormalized out.T [65, 256q]: row 64 = softmax sums
                    ps_o = psA.tile([65, 256], F32, tag="ps")
                    for ci in range(NCK):
                        nc.tensor.matmul(
                            ps_o[:],
                            v_gt[:, 260 * (wt * NCK + ci) + 65 * h:
                                 260 * (wt * NCK + ci) + 65 * (h + 1)],
                            ex[:, 256 * ci:256 * (ci + 1)],
                            start=(ci == 0), stop=(ci == NCK - 1))
                    rt = wkp.tile([1, 256], F32R, tag="rt")
                    nc.vector.reciprocal(rt[:], ps_o[64:65, :])
                    ps_b = psA.tile([64, 256], F32, tag="ps")
                    nc.tensor.matmul(ps_b[:], ones_c[:], rt[:],
                                     start=True, stop=True)
                    rb = wkp.tile([64, 256], F32R, tag="rb")
                    nc.vector.tensor_copy(rb[:], ps_b[:])
                    nc.vector.tensor_tensor(out=aoT[h][:], in0=ps_o[0:64, :],
                                            in1=rb[:], op=mybir.AluOpType.mult)
                for st in range(2):
                    ps_y = psA.tile([128, 256], F32, tag="ps")
                    for h in range(HEADS):
                        nc.tensor.matmul(ps_y[:],
                                         aoT[h][:, 128 * st:128 * (st + 1)],
                                         wp_t[h][:], start=(h == 0),
                                         stop=(h == HEADS - 1))
                    ysb = wkp.tile([128, 256], F32, tag="ysb")
                    nc.vector.tensor_tensor(out=ysb[:], in0=ps_y[:], in1=bp_b[:],
                                            op=mybir.AluOpType.add)
                    nc.sync.dma_start(out=yv[wt, 2 * st:2 * st + 2, :, :, :],
                                      in_=ysb[:])

    nc.compile()
    return nc


def _prep_inputs(x, w_qkv, b_qkv, w_proj, b_proj):
    x = np.asarray(x, np.float32)
    xp = np.zeros((D, H + 4, WP, C), np.float32)
    xp[:, 2:H + 2, 2:W + 2, :] = x[0]
    wq = np.ascontiguousarray(w_qkv[:, 0:C] * SCALE).astype(np.float32)
    wkk = np.ascontiguousarray(w_qkv[:, C:2 * C]).astype(np.float32)
    wv = np.ascontiguousarray(w_qkv[:, 2 * C:3 * C]).astype(np.float32)
    wpf = np.ascontiguousarray(np.asarray(w_proj, np.float32))
    bq = np.asarray(b_qkv, np.float32)
    bqkv_pack = np.zeros((128, 6), np.float32)
    bqkv_pack[:, 0] = bq[0:128] * SCALE
    bqkv_pack[:, 1] = bq[128:256] * SCALE
    bqkv_pack[:, 2] = bq[256:384]
    bqkv_pack[:, 3] = bq[384:512]
    bv = np.ascontiguousarray(bq[2 * C:3 * C].reshape(1, C)).astype(np.float32)
    bp = np.ascontiguousarray(np.asarray(b_proj, np.float32).reshape(1, C))

    # U: key-side indicators [64, (w', d, h)] over shard voxels
    U = np.zeros((64, WP, D, HH), np.float32)
    for d in range(D):
        U[d, :, d, :] = 1.0
    for r in range(HH):
        U[4 + r, :, :, r] = 1.0
    for wpp in range(WP):
        U[12 + wpp, wpp, :, :] = 1.0
    U = U.reshape(64, NV)

    in_maps = []
    for c in range(NCORES):
        xs = xp[:, 4 * c:4 * c + HH, :, :]         # [D, HH, WP, C] padded rows
        xT = np.ascontiguousarray(xs.reshape(NV, C).T)
        # query-ordered copy: columns in (wt, d, h own, wl) order
        xq = xs[:, 2:6, 2:2 + W, :]                # [D, 4, W, C]
        xq = xq.reshape(D, 4, NWT, 16, C).transpose(2, 0, 1, 3, 4)
        xTq = np.ascontiguousarray(xq.reshape(NQ, C).T)
        # V: query-side penalties, columns in (wt, d, hq, wl) order
        Vm = np.full((64, D, 4, W), -BIG, np.float32)
        for d in range(D):
            lo = min(max(d - 1, 0), D - KD)
            Vm[lo:lo + KD, d, :, :] = 0.0
        for hq in range(4):
            s = min(max(4 * c + hq - 2, 0), H - KH)
            for r in range(HH):
                if s <= 4 * c + r - 2 < s + KH:
                    Vm[4 + r, :, hq, :] = 0.0
        for wq_i in range(W):
            s = min(max(wq_i - 2, 0), W - KW)
            Vm[12 + s + 2:12 + s + 2 + KW, :, :, wq_i] = 0.0
        Vm = Vm.reshape(64, D, 4, NWT, 16).transpose(0, 3, 1, 2, 4)
        Vm = np.ascontiguousarray(Vm.reshape(64, NQ))
        ones1 = np.ones((1, 64), np.float32)
        ones60 = np.ones((128, 60), np.float32)
        in_maps.append({
            "xT": xT, "xTq": xTq, "wq": wq, "wk": wkk, "wv": wv, "wp": wpf,
            "u": U, "vq": Vm, "bqkv": bqkv_pack, "bv": bv, "bp": bp,
            "ones1": ones1, "ones60": ones60,
        })
    return in_maps


def kernel(x, w_qkv, b_qkv, w_proj, b_proj):
    if "nc" not in _CACHE:
        _CACHE["nc"] = _build_program()
    nc = _CACHE["nc"]
    in_maps = _prep_inputs(x, w_qkv, b_qkv, w_proj, b_proj)
    res = run_bass_kernel_spmd(nc, in_maps, list(range(NCORES)))
    out = np.zeros((1, D, H, W, C), np.float32)
    for c in range(NCORES):
        y = res.results[c]["y"].reshape(NWT, D, 4, 16, C)
        y = y.transpose(1, 2, 0, 3, 4).reshape(D, 4, W, C)
        out[0, :, 4 * c:4 * c + 4, :, :] = y
    return out



# revision 8
# speedup vs baseline: 1.1392x; 1.1392x over previous
"""Trainium2 Bass kernel for 3D neighborhood attention (NATTEN-style).

Sharding: H axis (32) split across 8 cores; each core owns 4 rows and stages
an 8-row halo window (host-padded). W padded by 2 each side (WP=52).

All-bf16 matmul pipeline (fp32 runs multi-pass on the real PE). Voxels are
staged KEY-MAJOR: vox(w,d,h) = w*32 + d*8 + h, so the k-projection writes kT
directly and score-chunk operands are strided AP slices (no data movement).

Neighborhood masking is folded into the score contraction via indicator /
penalty rows: scoresT[k,q] = k.T q + sum_r U[r,k] V[r,q], U = key-side h/w
position indicators (0/1), V = query-side -30000*(1-valid) penalties.
Contraction K = 64 (head dim) + 8 (h rows) + 52 (w rows) + 4 zero = 128.

Blocking: queries grouped into 6 blocks of 128 = (wt in 3) x (dg in 2) x
(d 2, h 4, w 16). The d-window for d in {2dg, 2dg+1} is exactly planes
[dg, dg+3) -- the chunk d-slice makes d-masking unnecessary. Keys per block:
20 w' x 3 d' x 8 h' = 480 = 4 chunks of 120 (partition dim 120, no padding).

Per block-headpair: scoresT chunks [120,128] -> PSUM [120,1024] -> exp (ACT,
bf16 out) -> AV (v_gt chunks [120,65], col 64 = ones giving softmax sums) ->
ps_o [65,512] (4 heads) -> one reciprocal_approx_fast [1,512] (DVE) ->
partition_broadcast (GPSIMD) -> one normalize-mult [64,4,128] writing packed
aoT -> proj (4 accumulating K=64 matmuls) -> y.
"""
import numpy as np
import ml_dtypes

import concourse.bass as bass
import concourse.bacc as bacc
import concourse.mybir as mybir
from concourse.tile import TileContext
from concourse.bass_utils import run_bass_kernel_spmd

BF16 = mybir.dt.bfloat16
F32 = mybir.dt.float32

NCORES = 8
D, H, W, C = 4, 32, 48, 256
HEADS, HD = 4, 64
SCALE = HD ** -0.5
BIG = 30000.0

HH = 8              # halo rows per core
WP = W + 4          # padded W
NV = WP * D * HH    # 1664 voxels per shard (key-major order w,d,h)
NQ = D * 4 * W      # 768 own queries per core
NWT = 3             # w tiles of 16 queries
NB = NWT * 2        # query blocks (wt, dg) of 128 queries
NCK = 4             # key chunks of 120 per block
CKK = 120           # keys per chunk (5 w' x 3 d' x 8 h')

_CACHE = {}


def _build_program():
    nc = bacc.Bacc("TRN2", target_bir_lowering=False, debug=False,
                   num_devices=NCORES)
    xT_in = nc.declare_dram_parameter("xT", [C, NV], BF16, isOutput=False)
    xTq_in = nc.declare_dram_parameter("xTq", [C, NQ], BF16, isOutput=False)
    wqkv_in = nc.declare_dram_parameter("wqkv", [C, 768], BF16, isOutput=False)
    wp_in = nc.declare_dram_parameter("wp", [C, C], BF16, isOutput=False)
    u_in = nc.declare_dram_parameter("u", [64, NV], BF16, isOutput=False)
    v_in = nc.declare_dram_parameter("vq", [64, NQ], BF16, isOutput=False)
    bqkv_in = nc.declare_dram_parameter("bqkv", [128, 4], F32, isOutput=False)
    bv_in = nc.declare_dram_parameter("bv", [1, C], F32, isOutput=False)
    bp_in = nc.declare_dram_parameter("bp", [1, C], F32, isOutput=False)
    y_out = nc.declare_dram_parameter("y", [NQ, C], F32, isOutput=True)
    v_dram = nc.dram_tensor("v_scratch", [NV, C], BF16)

    EXP = mybir.ActivationFunctionType.Exp
    ADD = mybir.AluOpType.add
    MUL = mybir.AluOpType.mult

    with TileContext(nc) as tc:
        with (
            nc.allow_low_precision(reason="bf16 pipeline, rel tol 2e-2"),
            tc.tile_pool(name="const", bufs=1) as cp,
            tc.tile_pool(name="psA", bufs=2, space="PSUM") as psA,
            tc.tile_pool(name="psS", bufs=2, space="PSUM") as psS,
            tc.tile_pool(name="psO", bufs=2, space="PSUM") as psO,
            tc.tile_pool(name="work", bufs=3) as wkp,
        ):
            # ---- constant / input loads ----
            xT = [cp.tile([128, NV], BF16, tag=f"xT{i}", name=f"xT{i}")
                  for i in range(2)]
            xTq = [cp.tile([128, NQ], BF16, tag=f"xTq{i}", name=f"xTq{i}")
                   for i in range(2)]
            wqkv = [cp.tile([128, 768], BF16, tag=f"wqkv{i}", name=f"wqkv{i}")
                    for i in range(2)]
            for i in range(2):
                nc.sync.dma_start(out=xT[i][:], in_=xT_in[128 * i:128 * (i + 1), :])
                nc.sync.dma_start(out=xTq[i][:], in_=xTq_in[128 * i:128 * (i + 1), :])
                nc.sync.dma_start(out=wqkv[i][:], in_=wqkv_in[128 * i:128 * (i + 1), :])
            wp_t = [cp.tile([64, C], BF16, tag=f"wp{h}", name=f"wp{h}")
                    for h in range(HEADS)]
            for h in range(HEADS):
                nc.scalar.dma_start(out=wp_t[h][:], in_=wp_in[64 * h:64 * (h + 1), :])
            # kTbig/qTbig: per-head column regions; rows 0:64 data, 64:128 mask
            kT = cp.tile([128, HEADS * NV], BF16, name="kTbig")
            qT = cp.tile([128, HEADS * NQ], BF16, name="qTbig")
            for h in range(HEADS):
                nc.gpsimd.dma_start(out=kT[64:128, h * NV:(h + 1) * NV], in_=u_in[:])
                nc.scalar.dma_start(out=qT[64:128, h * NQ:(h + 1) * NQ], in_=v_in[:])
            bqkv = cp.tile([128, 4], F32)
            nc.gpsimd.dma_start(out=bqkv[:], in_=bqkv_in[:])
            bv_row = cp.tile([1, C], F32)
            bp_row = cp.tile([1, C], F32)
            nc.gpsimd.dma_start(out=bv_row[:], in_=bv_in[:])
            nc.gpsimd.dma_start(out=bp_row[:], in_=bp_in[:])
            bv_b = cp.tile([128, C], F32)
            bp_b = cp.tile([128, C], F32)
            nc.gpsimd.partition_broadcast(bv_b[:], bv_row[:])
            nc.gpsimd.partition_broadcast(bp_b[:], bp_row[:])
            # v_gt: gathered v per (block, chunk): [120, 4 heads x 65]; col
            # 65h+64 = ones (softmax sum row of the AV matmul)
            v_gt = cp.tile([128, NB * NCK * 260], BF16, name="v_gt")
            ones_ap = v_gt[:].rearrange("p (k hh c) -> p k hh c",
                                        k=NB * NCK, hh=HEADS, c=65)[:, :, :, 64]
            nc.gpsimd.memset(ones_ap, 1.0)

            # ---- v projection (vox-major == key-major rows) ----
            vsb = cp.tile([128, 13 * C], BF16, name="vsb")
            for t in range(13):
                ps = psA.tile([128, C], F32, tag="ps")
                for kc in range(2):
                    nc.tensor.matmul(ps[:], xT[kc][:, 128 * t:128 * (t + 1)],
                                     wqkv[kc][:, 512:768], start=(kc == 0),
                                     stop=(kc == 1))
                nc.vector.tensor_tensor(out=vsb[:, C * t:C * (t + 1)], in0=ps[:],
                                        in1=bv_b[:], op=ADD)
            vdv = v_dram.ap().rearrange("(t p) c -> p t c", t=13, p=128)
            vsv = vsb[:].rearrange("p (t c) -> p t c", t=13)
            for i in range(4):
                sl = slice(*((0, 4), (4, 8), (8, 11), (11, 13))[i])
                nc.sync.dma_start(out=vdv[:, sl, :], in_=vsv[:, sl, :])

            # ---- gather v into per-(block, chunk) key order ----
            vv = v_dram.ap().rearrange("(w d h) c -> w d h c", w=WP, d=D, h=HH)
            vv = vv.rearrange("w d h (hh c) -> w d h hh c", hh=HEADS, c=64)
            gtv = v_gt[:].rearrange("p (k hh c) -> p k hh c",
                                    k=NB * NCK, hh=HEADS, c=65)
            for b in range(NB):
                wt, dg = b // 2, b % 2
                for ci in range(NCK):
                    eng = (nc.gpsimd, nc.sync, nc.gpsimd, nc.scalar)[ci]
                    w0 = 16 * wt + 5 * ci
                    eng.dma_start(
                        out=gtv[0:CKK, b * NCK + ci, :, 0:64],
                        in_=vv[w0:w0 + 5, dg:dg + 3, :, :, :])

            # ---- k projection (writes kT directly: key-major) ----
            for m in range(2):
                for nn in range(4):
                    ps = psA.tile([128, 416], F32, tag="ps")
                    for kc in range(2):
                        nc.tensor.matmul(
                            ps[:], wqkv[kc][:, 256 + 128 * m:256 + 128 * (m + 1)],
                            xT[kc][:, 416 * nn:416 * (nn + 1)],
                            start=(kc == 0), stop=(kc == 1))
                    for hh in range(2):
                        dst = kT[0:64, (2 * m + hh) * NV + 416 * nn:
                                 (2 * m + hh) * NV + 416 * (nn + 1)]
                        src = ps[64 * hh:64 * (hh + 1), :]
                        sc = bqkv[64 * hh:64 * (hh + 1), 2 + m:3 + m]
                        if hh == 0:
                            nc.vector.tensor_scalar(out=dst, in0=src, scalar1=sc,
                                                    scalar2=None, op0=ADD)
                        else:
                            nc.scalar.activation(
                                dst, src, mybir.ActivationFunctionType.Identity,
                                bias=sc)

            # ---- q projection (query-block order) ----
            for m in range(2):
                for nn in range(2):
                    ps = psA.tile([128, 384], F32, tag="ps")
                    for kc in range(2):
                        nc.tensor.matmul(ps[:],
                                         wqkv[kc][:, 128 * m:128 * (m + 1)],
                                         xTq[kc][:, 384 * nn:384 * (nn + 1)],
                                         start=(kc == 0), stop=(kc == 1))
                    for hh in range(2):
                        nc.vector.tensor_scalar(
                            out=qT[0:64, (2 * m + hh) * NQ + 384 * nn:
                                 (2 * m + hh) * NQ + 384 * (nn + 1)],
                            in0=ps[64 * hh:64 * (hh + 1), :],
                            scalar1=bqkv[64 * hh:64 * (hh + 1), m:m + 1],
                            scalar2=None, op0=ADD)

            # ---- duplicate kT into per-dg d-sliced copies (contiguous
            # chunks: matmul stationary APs must collapse to one free dim) ----
            NV2 = WP * 3 * HH  # 1248
            kT2 = cp.tile([128, HEADS * 2 * NV2], BF16, name="kT2")
            kv4 = kT[:].rearrange("p (hh w d e) -> p hh w d e",
                                  hh=HEADS, w=WP, d=D, e=HH)
            for h in range(HEADS):
                for dg in range(2):
                    eng = (nc.sync, nc.scalar, nc.gpsimd)[(2 * h + dg) % 3]
                    eng.dma_start(
                        out=kT2[:, (2 * h + dg) * NV2:(2 * h + dg + 1) * NV2],
                        in_=kv4[:, h, :, dg:dg + 3, :])

            # ---- attention + proj ----
            aoT = cp.tile([64, HEADS * NQ], BF16, name="aoT")
            aov = aoT[:].rearrange("p (hh q) -> p hh q", hh=HEADS)
            ysb = cp.tile([128, NB * C], F32, name="ysb")
            for b in range(NB):
                wt, dg = b // 2, b % 2
                ps_o = psO.tile([65, 512], F32, tag="ps_o")
                for hp in range(2):
                    ps_s = psS.tile([128, 1024], F32, tag="ps_s")
                    for h2 in range(2):
                        h = 2 * hp + h2
                        base = (2 * h + dg) * NV2
                        for ci in range(NCK):
                            c0 = base + (16 * wt + 5 * ci) * 24
                            nc.tensor.matmul(
                                ps_s[0:CKK, 512 * h2 + 128 * ci:
                                     512 * h2 + 128 * (ci + 1)],
                                kT2[:, c0:c0 + CKK],
                                qT[:, h * NQ + 128 * b:h * NQ + 128 * (b + 1)],
                                start=True, stop=True)
                    ex = wkp.tile([128, 1024], BF16, tag="ex")
                    nc.scalar.activation(ex[0:CKK, :], ps_s[0:CKK, :], EXP)
                    for h2 in range(2):
                        h = 2 * hp + h2
                        for ci in range(NCK):
                            nc.tensor.matmul(
                                ps_o[:, 128 * h:128 * (h + 1)],
                                v_gt[0:CKK, (b * NCK + ci) * 260 + 65 * h:
                                     (b * NCK + ci) * 260 + 65 * (h + 1)],
                                ex[0:CKK, 512 * h2 + 128 * ci:
                                   512 * h2 + 128 * (ci + 1)],
                                start=(ci == 0), stop=(ci == NCK - 1))
                st = wkp.tile([1, 512], F32, tag="st")
                nc.scalar.activation(st[:], ps_o[64:65, :],
                                     mybir.ActivationFunctionType.Copy)
                rt = wkp.tile([1, 512], F32, tag="rt")
                nc.vector.reciprocal_approx_fast(rt[:], st[:])
                rb = wkp.tile([64, 512], F32, tag="rb")
                nc.gpsimd.partition_broadcast(rb[:], rt[:])
                nc.vector.tensor_tensor(
                    out=aov[:, :, 128 * b:128 * (b + 1)],
                    in0=ps_o[0:64, :].rearrange("p (hh q) -> p hh q", hh=HEADS),
                    in1=rb[:].rearrange("p (hh q) -> p hh q", hh=HEADS),
                    op=MUL)
                ps_y = psA.tile([128, C], F32, tag="ps")
                for h in range(HEADS):
                    nc.tensor.matmul(ps_y[:],
                                     aoT[:, h * NQ + 128 * b:h * NQ + 128 * (b + 1)],
                                     wp_t[h][:], start=(h == 0),
                                     stop=(h == HEADS - 1))
                nc.vector.tensor_tensor(out=ysb[:, C * b:C * (b + 1)],
                                        in0=ps_y[:], in1=bp_b[:], op=ADD)
            yv = y_out.ap().rearrange("(b p) c -> p b c", b=NB, p=128)
            ysv = ysb[:].rearrange("p (b c) -> p b c", b=NB)
            nc.sync.dma_start(out=yv[:, 0:3, :], in_=ysv[:, 0:3, :])
            nc.sync.dma_start(out=yv[:, 3:6, :], in_=ysv[:, 3:6, :])

    nc.compile()
    return nc


def _prep_inputs(x, w_qkv, b_qkv, w_proj, b_proj):
    x = np.asarray(x, np.float32)
    xp = np.zeros((D, H + 4, WP, C), np.float32)
    xp[:, 2:H + 2, 2:W + 2, :] = x[0]
    wq = np.asarray(w_qkv[:, 0:C], np.float32) * SCALE
    wqkv_pack = np.concatenate(
        [wq, np.asarray(w_qkv[:, C:3 * C], np.float32)], axis=1)
    wqkv_pack = wqkv_pack.astype(ml_dtypes.bfloat16)
    wpf = np.asarray(w_proj, np.float32).astype(ml_dtypes.bfloat16)
    bq = np.asarray(b_qkv, np.float32)
    bqkv_pack = np.zeros((128, 4), np.float32)
    bqkv_pack[:, 0] = bq[0:128] * SCALE
    bqkv_pack[:, 1] = bq[128:256] * SCALE
    bqkv_pack[:, 2] = bq[256:384]
    bqkv_pack[:, 3] = bq[384:512]
    bv = np.ascontiguousarray(bq[2 * C:3 * C].reshape(1, C)).astype(np.float32)
    bp = np.ascontiguousarray(np.asarray(b_proj, np.float32).reshape(1, C))

    # U: key-side indicators [64, (w', d, h)]: rows 0-7 = h-halo row, rows
    # 8..59 = w' position, rows 60-63 zero
    U = np.zeros((64, WP, D, HH), np.float32)
    for r in range(HH):
        U[r, :, :, r] = 1.0
    for wpp in range(WP):
        U[8 + wpp, wpp, :, :] = 1.0
    U = U.reshape(64, NV).astype(ml_dtypes.bfloat16)

    in_maps = []
    for c in range(NCORES):
        xs = xp[:, 4 * c:4 * c + HH, :, :]            # [D, HH, WP, C]
        xk = np.ascontiguousarray(xs.transpose(2, 0, 1, 3))  # [WP, D, HH, C]
        xT = np.ascontiguousarray(xk.reshape(NV, C).T).astype(ml_dtypes.bfloat16)
        # query order: (wt, dg, dl, hl, wl)
        xq = xs[:, 2:6, 2:2 + W, :]                   # [D, 4, W, C]
        xq = xq.reshape(2, 2, 4, NWT, 16, C)          # [dg, dl, hl, wt, wl, C]
        xq = xq.transpose(3, 0, 1, 2, 4, 5)           # [wt, dg, dl, hl, wl, C]
        xTq = np.ascontiguousarray(
            xq.reshape(NQ, C).T).astype(ml_dtypes.bfloat16)
        # V: query-side penalties [64, NQ] in block order
        Vm = np.full((64, NWT, 2, 2, 4, 16), -BIG, np.float32)
        Vm[60:64] = 0.0
        for hl in range(4):
            hg = 4 * c + hl
            s = min(max(hg - 2, 0), H - 5)
            for r in range(HH):
                if s <= 4 * c + r - 2 < s + 5:
                    Vm[r, :, :, :, hl, :] = 0.0
        for wt in range(NWT):
            for wl in range(16):
                wg = 16 * wt + wl
                s = min(max(wg - 2, 0), W - 5)
                Vm[8 + s + 2:8 + s + 7, wt, :, :, :, wl] = 0.0
        Vm = np.ascontiguousarray(
            Vm.reshape(64, NQ)).astype(ml_dtypes.bfloat16)
        in_maps.append({
            "xT": xT, "xTq": xTq, "wqkv": wqkv_pack, "wp": wpf,
            "u": U, "vq": Vm, "bqkv": bqkv_pack, "bv": bv, "bp": bp,
        })
    return in_maps


def kernel(x, w_qkv, b_qkv, w_proj, b_proj):
    if "nc" not in _CACHE:
        _CACHE["nc"] = _build_program()
    nc = _CACHE["nc"]
    in_maps = _prep_inputs(x, w_qkv, b_qkv, w_proj, b_proj)
    res = run_bass_kernel_spmd(nc, in_maps, list(range(NCORES)))
    out = np.zeros((1, D, H, W, C), np.float32)
    for c in range(NCORES):
        y = res.results[c]["y"].reshape(NWT, 2, 2, 4, 16, C)
        for wt in range(NWT):
            for dg in range(2):
                for dl in range(2):
                    out[0, 2 * dg + dl, 4 * c:4 * c + 4,
                        16 * wt:16 * (wt + 1), :] = y[wt, dg, dl]
    return out


# revision 15
# speedup vs baseline: 1.4407x; 1.2646x over previous
"""Trainium2 Bass kernel for 3D neighborhood attention (NATTEN-style).

Sharding: H axis (32) split across 8 cores; each core owns 4 rows and stages
an 8-row halo window (host-padded). W padded by 2 each side (WP=52).

All-bf16 matmul pipeline (fp32 runs multi-pass on the real PE). Voxels are
staged KEY-MAJOR: vox(w,d,h) = w*32 + d*8 + h, so the k-projection writes kT
directly and score-chunk operands are strided AP slices (no data movement).

Neighborhood masking is folded into the score contraction via indicator /
penalty rows: scoresT[k,q] = k.T q + sum_r U[r,k] V[r,q], U = key-side h/w
position indicators (0/1), V = query-side -30000*(1-valid) penalties.
Contraction K = 64 (head dim) + 8 (h rows) + 52 (w rows) + 4 zero = 128.

Blocking: queries grouped into 6 blocks of 128 = (wt in 3) x (dg in 2) x
(d 2, h 4, w 16). The d-window for d in {2dg, 2dg+1} is exactly planes
[dg, dg+3) -- the chunk d-slice makes d-masking unnecessary. Keys per block:
20 w' x 3 d' x 8 h' = 480 = 4 chunks of 120 (partition dim 120, no padding).

Per block-headpair: scoresT chunks [120,128] -> PSUM [120,1024] -> exp (ACT,
bf16 out) -> AV (v_gt chunks [120,65], col 64 = ones giving softmax sums) ->
ps_o [65,512] (4 heads) -> one reciprocal_approx_fast [1,512] (DVE) ->
partition_broadcast (GPSIMD) -> one normalize-mult [64,4,128] writing packed
aoT -> proj (4 accumulating K=64 matmuls) -> y.
"""
import numpy as np
import ml_dtypes

import concourse.bass as bass
import concourse.bacc as bacc
import concourse.mybir as mybir
from concourse.tile import TileContext
from concourse.bass_utils import run_bass_kernel_spmd

BF16 = mybir.dt.bfloat16
F32 = mybir.dt.float32

NCORES = 8
D, H, W, C = 4, 32, 48, 256
HEADS, HD = 4, 64
SCALE = HD ** -0.5
BIG = 30000.0

HH = 8              # halo rows per core
WP = W + 4          # padded W
NV = WP * D * HH    # 1664 voxels per shard (key-major order w,d,h)
NQ = D * 4 * W      # 768 own queries per core
NWT = 3             # w tiles of 16 queries
NB = NWT * 2        # query blocks (wt, dg) of 128 queries
NCK = 4             # key chunks of 120 per block
CKK = 120           # keys per chunk (5 w' x 3 d' x 8 h')

_CACHE = {}


def _build_program():
    nc = bacc.Bacc("TRN2", target_bir_lowering=False, debug=False,
                   num_devices=NCORES)
    xT_in = nc.declare_dram_parameter("xT", [C, NV], BF16, isOutput=False)
    xTq_in = nc.declare_dram_parameter("xTq", [C, NQ], BF16, isOutput=False)
    wqkv_in = nc.declare_dram_parameter("wqkv", [C, 768], BF16, isOutput=False)
    wp_in = nc.declare_dram_parameter("wp", [C, C], BF16, isOutput=False)
    NV2 = WP * 3 * HH  # 1248: voxels of one dg d-slice, (w, d', h) order
    u_in = nc.declare_dram_parameter("u", [64, 2 * NV2], BF16, isOutput=False)
    v_in = nc.declare_dram_parameter("vq", [64, NQ], BF16, isOutput=False)
    bqkv_in = nc.declare_dram_parameter("bqkv", [128, 4], F32, isOutput=False)
    bv_in = nc.declare_dram_parameter("bv", [1, C], F32, isOutput=False)
    bp_in = nc.declare_dram_parameter("bp", [1, C], F32, isOutput=False)
    y_out = nc.declare_dram_parameter("y", [NQ, C], F32, isOutput=True)
    v_dram = nc.dram_tensor("v_scratch", [NV, C], BF16)

    EXP = mybir.ActivationFunctionType.Exp
    ADD = mybir.AluOpType.add
    MUL = mybir.AluOpType.mult

    with TileContext(nc) as tc:
        with (
            nc.allow_low_precision(reason="bf16 pipeline, rel tol 2e-2"),
            tc.tile_pool(name="const", bufs=1) as cp,
            tc.tile_pool(name="psA", bufs=2, space="PSUM") as psA,
            tc.tile_pool(name="psS", bufs=2, space="PSUM") as psS,
            tc.tile_pool(name="psO", bufs=2, space="PSUM") as psO,
            tc.tile_pool(name="work", bufs=3) as wkp,
        ):
            # ---- constant / input loads (xT + wqkv first: they gate PE) ----
            xT = [cp.tile([128, NV], BF16, tag=f"xT{i}", name=f"xT{i}")
                  for i in range(2)]
            xTq = [cp.tile([128, NQ], BF16, tag=f"xTq{i}", name=f"xTq{i}")
                   for i in range(2)]
            wqkv = [cp.tile([128, 768], BF16, tag=f"wqkv{i}", name=f"wqkv{i}")
                    for i in range(2)]
            nc.sync.dma_start(out=xT[0][:], in_=xT_in[0:128, :])
            nc.scalar.dma_start(out=xT[1][:], in_=xT_in[128:256, :])
            nc.sync.dma_start(out=wqkv[0][:], in_=wqkv_in[0:128, :])
            nc.scalar.dma_start(out=wqkv[1][:], in_=wqkv_in[128:256, :])
            nc.sync.dma_start(out=xTq[0][:], in_=xTq_in[0:128, :])
            nc.scalar.dma_start(out=xTq[1][:], in_=xTq_in[128:256, :])
            wp_t = [cp.tile([64, C], BF16, tag=f"wp{h}", name=f"wp{h}")
                    for h in range(HEADS)]
            for h in range(HEADS):
                nc.gpsimd.dma_start(out=wp_t[h][:], in_=wp_in[64 * h:64 * (h + 1), :])
            # kT2: per-(head, dg) regions [(w, d', h)] ; qT: per-head regions.
            # rows 0:64 data, 64:128 mask (U / V penalty rows)
            kT2 = cp.tile([128, HEADS * 2 * NV2], BF16, name="kT2")
            qT = cp.tile([128, HEADS * NQ], BF16, name="qTbig")
            for h in range(HEADS):
                eng = (nc.sync, nc.scalar, nc.gpsimd)[h % 3]
                eng.dma_start(out=kT2[64:128, 2 * h * NV2:(2 * h + 2) * NV2],
                              in_=u_in[:])
                eng.dma_start(out=qT[64:128, h * NQ:(h + 1) * NQ], in_=v_in[:])
            bqkv = cp.tile([128, 4], F32)
            nc.gpsimd.dma_start(out=bqkv[:], in_=bqkv_in[:])
            bv_row = cp.tile([1, C], F32)
            bp_row = cp.tile([1, C], F32)
            nc.gpsimd.dma_start(out=bv_row[:], in_=bv_in[:])
            nc.gpsimd.dma_start(out=bp_row[:], in_=bp_in[:])
            bv_b = cp.tile([128, C], F32)
            bp_b = cp.tile([128, C], F32)
            nc.gpsimd.partition_broadcast(bv_b[:], bv_row[:])
            nc.gpsimd.partition_broadcast(bp_b[:], bp_row[:])
            # v_gt: gathered v per (block, chunk): [120, 4 heads x 65]; col
            # 65h+64 = ones (softmax sum row of the AV matmul)
            v_gt = cp.tile([128, NB * NCK * 260], BF16, name="v_gt")
            ones_ap = v_gt[:].rearrange("p (k hh c) -> p k hh c",
                                        k=NB * NCK, hh=HEADS, c=65)[:, :, :, 64]
            nc.gpsimd.memset(ones_ap, 1.0)

            # ---- v projection (vox-major == key-major rows) ----
            vsb = cp.tile([128, 13 * C], BF16, name="vsb")

            def v_proj():
                for t in range(13):
                    ps = psA.tile([128, C], F32, tag="ps")
                    for kc in range(2):
                        nc.tensor.matmul(ps[:], xT[kc][:, 128 * t:128 * (t + 1)],
                                         wqkv[kc][:, 512:768], start=(kc == 0),
                                         stop=(kc == 1))
                    nc.vector.tensor_tensor(out=vsb[:, C * t:C * (t + 1)],
                                            in0=ps[:], in1=bv_b[:], op=ADD)
                vdv = v_dram.ap().rearrange("(t p) c -> p t c", t=13, p=128)
                vsv = vsb[:].rearrange("p (t c) -> p t c", t=13)
                for i in range(4):
                    sl = slice(*((0, 4), (4, 8), (8, 11), (11, 13))[i])
                    nc.sync.dma_start(out=vdv[:, sl, :], in_=vsv[:, sl, :])

            # gather v into per-(block, chunk) key order
            def v_gather():
                vv = v_dram.ap().rearrange("(w d h) c -> w d h c",
                                           w=WP, d=D, h=HH)
                vv = vv.rearrange("w d h (hh c) -> w d h hh c", hh=HEADS, c=64)
                gtv = v_gt[:].rearrange("p (k hh c) -> p k hh c",
                                        k=NB * NCK, hh=HEADS, c=65)
                for b in range(NB):
                    wt, dg = b // 2, b % 2
                    for ci in range(NCK):
                        eng = (nc.gpsimd, nc.sync, nc.gpsimd, nc.scalar)[ci]
                        w0 = 16 * wt + 5 * ci
                        eng.dma_start(
                            out=gtv[0:CKK, b * NCK + ci, :, 0:64],
                            in_=vv[w0:w0 + 5, dg:dg + 3, :, :, :])

            # ---- k / q projections (k writes dg-duplicated kT2 directly) ----
            def k_proj(m):
                for nn in range(4):
                    ps = psA.tile([128, 416], F32, tag="ps")
                    for kc in range(2):
                        nc.tensor.matmul(
                            ps[:], wqkv[kc][:, 256 + 128 * m:256 + 128 * (m + 1)],
                            xT[kc][:, 416 * nn:416 * (nn + 1)],
                            start=(kc == 0), stop=(kc == 1))
                    for hh in range(2):
                        h = 2 * m + hh
                        src = ps[64 * hh:64 * (hh + 1), :].rearrange(
                            "p (w d e) -> p w d e", w=13, d=D, e=HH)
                        sc = bqkv[64 * hh:64 * (hh + 1), 2 + m:3 + m]
                        for dg in range(2):
                            dst = kT2[0:64, (2 * h + dg) * NV2 + 312 * nn:
                                      (2 * h + dg) * NV2 + 312 * (nn + 1)]
                            srcd = src[:, :, dg:dg + 3, :]
                            if dg == 0:
                                nc.vector.tensor_scalar(
                                    out=dst, in0=srcd, scalar1=sc,
                                    scalar2=None, op0=ADD)
                            else:
                                nc.scalar.activation(
                                    dst, srcd,
                                    mybir.ActivationFunctionType.Identity,
                                    bias=sc)

            def q_proj(m):
                for nn in range(2):
                    ps = psA.tile([128, 384], F32, tag="ps")
                    for kc in range(2):
                        nc.tensor.matmul(ps[:],
                                         wqkv[kc][:, 128 * m:128 * (m + 1)],
                                         xTq[kc][:, 384 * nn:384 * (nn + 1)],
                                         start=(kc == 0), stop=(kc == 1))
                    for hh in range(2):
                        nc.vector.tensor_scalar(
                            out=qT[0:64, (2 * m + hh) * NQ + 384 * nn:
                                 (2 * m + hh) * NQ + 384 * (nn + 1)],
                            in0=ps[64 * hh:64 * (hh + 1), :],
                            scalar1=bqkv[64 * hh:64 * (hh + 1), m:m + 1],
                            scalar2=None, op0=ADD)

            k_proj(0)
            q_proj(0)
            v_proj()
            v_gather()
            k_proj(1)
            q_proj(1)

            # ---- attention + proj ----
            aoT = cp.tile([64, HEADS * NQ], BF16, name="aoT")
            aov = aoT[:].rearrange("p (hh q) -> p hh q", hh=HEADS)
            ysb = cp.tile([128, NB * C], F32, name="ysb")
            for b in range(NB):
                wt, dg = b // 2, b % 2
                ps_o = psO.tile([65, 512], F32, tag="ps_o")
                for hp in range(2):
                    ps_s = psS.tile([128, 1024], F32, tag="ps_s")
                    for h2 in range(2):
                        h = 2 * hp + h2
                        base = (2 * h + dg) * NV2
                        for ci in range(NCK):
                            c0 = base + (16 * wt + 5 * ci) * 24
                            nc.tensor.matmul(
                                ps_s[0:CKK, 512 * h2 + 128 * ci:
                                     512 * h2 + 128 * (ci + 1)],
                                kT2[:, c0:c0 + CKK],
                                qT[:, h * NQ + 128 * b:h * NQ + 128 * (b + 1)],
                                start=True, stop=True)
                    ex = wkp.tile([128, 1024], BF16, tag="ex")
                    nc.scalar.activation(ex[0:CKK, :], ps_s[0:CKK, :], EXP)
                    for h2 in range(2):
                        h = 2 * hp + h2
                        for ci in range(NCK):
                            nc.tensor.matmul(
                                ps_o[:, 128 * h:128 * (h + 1)],
                                v_gt[0:CKK, (b * NCK + ci) * 260 + 65 * h:
                                     (b * NCK + ci) * 260 + 65 * (h + 1)],
                                ex[0:CKK, 512 * h2 + 128 * ci:
                                   512 * h2 + 128 * (ci + 1)],
                                start=(ci == 0), stop=(ci == NCK - 1))
                st = wkp.tile([1, 512], F32, tag="st")
                nc.scalar.activation(st[:], ps_o[64:65, :],
                                     mybir.ActivationFunctionType.Copy)
                rt = wkp.tile([1, 512], F32, tag="rt")
                nc.vector.reciprocal_approx_fast(rt[:], st[:])
                rb = wkp.tile([64, 512], F32, tag="rb")
                nc.gpsimd.partition_broadcast(rb[:], rt[:])
                nc.vector.tensor_tensor(
                    out=aov[:, :, 128 * b:128 * (b + 1)],
                    in0=ps_o[0:64, :].rearrange("p (hh q) -> p hh q", hh=HEADS),
                    in1=rb[:].rearrange("p (hh q) -> p hh q", hh=HEADS),
                    op=MUL)
                ps_y = psA.tile([128, C], F32, tag="ps")
                for h in range(HEADS):
                    nc.tensor.matmul(ps_y[:],
                                     aoT[:, h * NQ + 128 * b:h * NQ + 128 * (b + 1)],
                                     wp_t[h][:], start=(h == 0),
                                     stop=(h == HEADS - 1))
                nc.vector.tensor_tensor(out=ysb[:, C * b:C * (b + 1)],
                                        in0=ps_y[:], in1=bp_b[:], op=ADD)
            yv = y_out.ap().rearrange("(b p) c -> p b c", b=NB, p=128)
            ysv = ysb[:].rearrange("p (b c) -> p b c", b=NB)
            nc.sync.dma_start(out=yv[:, 0:3, :], in_=ysv[:, 0:3, :])
            nc.sync.dma_start(out=yv[:, 3:6, :], in_=ysv[:, 3:6, :])

    nc.compile()
    return nc


def _prep_inputs(x, w_qkv, b_qkv, w_proj, b_proj):
    x = np.asarray(x, np.float32)
    xp = np.zeros((D, H + 4, WP, C), np.float32)
    xp[:, 2:H + 2, 2:W + 2, :] = x[0]
    wq = np.asarray(w_qkv[:, 0:C], np.float32) * SCALE
    wqkv_pack = np.concatenate(
        [wq, np.asarray(w_qkv[:, C:3 * C], np.float32)], axis=1)
    wqkv_pack = wqkv_pack.astype(ml_dtypes.bfloat16)
    wpf = np.asarray(w_proj, np.float32).astype(ml_dtypes.bfloat16)
    bq = np.asarray(b_qkv, np.float32)
    bqkv_pack = np.zeros((128, 4), np.float32)
    bqkv_pack[:, 0] = bq[0:128] * SCALE
    bqkv_pack[:, 1] = bq[128:256] * SCALE
    bqkv_pack[:, 2] = bq[256:384]
    bqkv_pack[:, 3] = bq[384:512]
    bv = np.ascontiguousarray(bq[2 * C:3 * C].reshape(1, C)).astype(np.float32)
    bp = np.ascontiguousarray(np.asarray(b_proj, np.float32).reshape(1, C))

    # U: key-side indicators [64, (w', d, h)]: rows 0-7 = h-halo row, rows
    # 8..59 = w' position, rows 60-63 zero
    U = np.zeros((64, WP, D, HH), np.float32)
    for r in range(HH):
        U[r, :, :, r] = 1.0
    for wpp in range(WP):
        U[8 + wpp, wpp, :, :] = 1.0
    NV2 = WP * 3 * HH
    U2 = np.concatenate(
        [U[:, :, 0:3, :].reshape(64, NV2), U[:, :, 1:4, :].reshape(64, NV2)],
        axis=1).astype(ml_dtypes.bfloat16)

    in_maps = []
    for c in range(NCORES):
        xs = xp[:, 4 * c:4 * c + HH, :, :]            # [D, HH, WP, C]
        xk = np.ascontiguousarray(xs.transpose(2, 0, 1, 3))  # [WP, D, HH, C]
        xT = np.ascontiguousarray(xk.reshape(NV, C).T).astype(ml_dtypes.bfloat16)
        # query order: (wt, dg, dl, hl, wl)
        xq = xs[:, 2:6, 2:2 + W, :]                   # [D, 4, W, C]
        xq = xq.reshape(2, 2, 4, NWT, 16, C)          # [dg, dl, hl, wt, wl, C]
        xq = xq.transpose(3, 0, 1, 2, 4, 5)           # [wt, dg, dl, hl, wl, C]
        xTq = np.ascontiguousarray(
            xq.reshape(NQ, C).T).astype(ml_dtypes.bfloat16)
        # V: query-side penalties [64, NQ] in block order
        Vm = np.full((64, NWT, 2, 2, 4, 16), -BIG, np.float32)
        Vm[60:64] = 0.0
        for hl in range(4):
            hg = 4 * c + hl
            s = min(max(hg - 2, 0), H - 5)
            for r in range(HH):
                if s <= 4 * c + r - 2 < s + 5:
                    Vm[r, :, :, :, hl, :] = 0.0
        for wt in range(NWT):
            for wl in range(16):
                wg = 16 * wt + wl
                s = min(max(wg - 2, 0), W - 5)
                Vm[8 + s + 2:8 + s + 7, wt, :, :, :, wl] = 0.0
        Vm = np.ascontiguousarray(
            Vm.reshape(64, NQ)).astype(ml_dtypes.bfloat16)
        in_maps.append({
            "xT": xT, "xTq": xTq, "wqkv": wqkv_pack, "wp": wpf,
            "u": U2, "vq": Vm, "bqkv": bqkv_pack, "bv": bv, "bp": bp,
        })
    return in_maps


def kernel(x, w_qkv, b_qkv, w_proj, b_proj):
    if "nc" not in _CACHE:
        _CACHE["nc"] = _build_program()
    nc = _CACHE["nc"]
    in_maps = _prep_inputs(x, w_qkv, b_qkv, w_proj, b_proj)
    res = run_bass_kernel_spmd(nc, in_maps, list(range(NCORES)))
    out = np.zeros((1, D, H, W, C), np.float32)
    for c in range(NCORES):
        y = res.results[c]["y"].reshape(NWT, 2, 2, 4, 16, C)
        for wt in range(NWT):
            for dg in range(2):
                for dl in range(2):
                    out[0, 2 * dg + dl, 4 * c:4 * c + 4,
                        16 * wt:16 * (wt + 1), :] = y[wt, dg, dl]
    return out
